# revision 2
# baseline (speedup 1.0000x reference)
"""GroupDRO segment-reduce kernel for 8 Trainium2 NeuronCores (v2).

Step-mask difference algorithm:
  - Shard the 2^24 elements across 8 cores (2M each), laid out [128, 16384].
  - Decompose g = hi*128 + lo (hi in [0,79), lo in [0,128)).
  - Pack v = hi + 0.9*loss in fp16 (exact integer part; loss quantized).
  - Per chunk (one SBUF column of 128 samples), build from a shared affine
    tile v_hD[j] = v - j (j = 0..78):
      S[j]  = clamp01(v - j)        -> contributes cnt{hi>j} + 0.9*loss at hi=j
      C'[j] = sign(v - j + 0.05)    -> 2*cnt{hi>=j} - total  ({-1,+1} convention)
    plus a 128-wide one-hot of lo as the stationary operand.
  - One 158-column matmul per chunk accumulates psum[lo, 0:79]  = sum oh_lo*C',
    psum[lo, 79:158] = sum oh_lo*S.
  - AllReduce the [128, 158] partials, then decode:
      cntge[j] = (C'[j] + total)/2 ;  counts[h] = cntge[h] - cntge[h+1]
      sums[h]  = (S[h] - cntge[h+1]) / 0.9
    and finish mean / exp-weight / weighted-sum on device.
"""

import os
import sys

import numpy as np

sys.path.insert(0, "/opt/trn_rl_repo")

import concourse.bacc as bacc
import concourse.tile as tile
from concourse import mybir
from concourse.bass_utils import run_bass_kernel_spmd

NUM_GROUPS = 10000
STEP_SIZE = 0.01
B = 16777216
NCORES = 8
P = 128
HI_W = 79                 # hi in [0, 79)
MOV_W = 2 * HI_W          # [C' | S]
LOSS_SCALE = 0.9
E_PER_CORE = B // NCORES  # 2,097,152
FREE = E_PER_CORE // P    # 16384 columns per core
FT = 256                  # columns per DMA tile
FC = 128                  # columns per build sub-batch

F32 = mybir.dt.float32
I32 = mybir.dt.int32
F16 = mybir.dt.float16

AluOp = mybir.AluOpType
ActFn = mybir.ActivationFunctionType


def _fill_steps(width):
    """Doubling plan covering [1, width): list of (src_width, dst_offset)."""
    steps = []
    filled = 1
    while filled < width:
        n = min(filled, width - filled)
        steps.append((n, filled))
        filled += n
    return steps


def _build_program(free=FREE, ft=FT, fc=FC, ncores=NCORES, debug=False):
    nc = bacc.Bacc("TRN2", target_bir_lowering=False, debug=debug,
                   num_devices=ncores)

    losses_d = nc.dram_tensor("losses", [P, free], F32, kind="ExternalInput")
    gids_d = nc.dram_tensor("gids", [P, free], I32, kind="ExternalInput")
    gw_d = nc.dram_tensor("gw", [P, HI_W], F32, kind="ExternalInput")
    out_d = nc.dram_tensor("out", [1, 1], F32, kind="ExternalOutput")

    n_tiles = free // ft
    n_sub = ft // fc

    # engine split knobs (sim-tuned): which engine runs each v_hD fill step
    # P=Pool(gpsimd), A=ACT(scalar), D=DVE(vector); steps are
    # (1,1),(2,2),(4,4),(8,8),(16,16),(32,32),(15,64) for width 79.
    vh_plan = os.environ.get("K_VH_PLAN", "PPPPDDA")
    lod_bufs = int(os.environ.get("K_LOD_BUFS", "1"))
    build_bufs = int(os.environ.get("K_BUILD_BUFS", "2"))
    vh_bufs = int(os.environ.get("K_VH_BUFS", "2"))
    # how many C' columns ACT computes via Sign (rest: DVE is_ge {0,1})
    c_act = int(os.environ.get("K_C_ACT", str(HI_W)))
    lod_eng = os.environ.get("K_LOD_ENG", "D")  # lo_D fill engine
    prep_eng = os.environ.get("K_PREP_ENG", "P")

    with tile.TileContext(nc) as tc:
        with (
            tc.tile_pool(name="const", bufs=1) as cpool,
            tc.tile_pool(name="inp", bufs=int(os.environ.get("K_INP_BUFS", "2"))) as ipool,
            tc.tile_pool(name="prep", bufs=int(os.environ.get("K_PREP_BUFS", "2"))) as qpool,
            tc.tile_pool(name="vh", bufs=vh_bufs) as vpool,
            tc.tile_pool(name="lod", bufs=lod_bufs) as lpool,
            tc.tile_pool(name="build", bufs=build_bufs) as bpool,
            tc.tile_pool(name="fin", bufs=1) as fpool,
            tc.tile_pool(name="psum", bufs=1, space="PSUM") as ppool,
            tc.tile_pool(name="dram", bufs=1, space="DRAM") as drpool,
        ):
            psum_acc = ppool.tile([P, MOV_W], F32, space="PSUM")

            sign_bias = cpool.tile([P, 1], F32, tag="sign_bias")
            nc.vector.memset(sign_bias[:], 0.05)

            def eng(ch):
                return {"P": nc.gpsimd, "A": nc.scalar, "D": nc.vector}[ch]

            prime_fc = int(os.environ.get("K_PRIME_FC", "128"))
            first = True
            for t in range(n_tiles):
                sl = slice(t * ft, (t + 1) * ft)
                fc_t = prime_fc if (t == 0 and prime_fc) else fc
                n_sub_t = ft // fc_t
                L32 = ipool.tile([P, ft], F32, tag="L32")
                G32 = ipool.tile([P, ft], I32, tag="G32")
                nc.sync.dma_start(out=L32[:], in_=losses_d.ap()[:, sl])
                nc.sync.dma_start(out=G32[:], in_=gids_d.ap()[:, sl])

                pe = eng("D" if t == 0 else prep_eng)
                hi32 = qpool.tile([P, ft], I32, tag="hi32")
                lo32 = qpool.tile([P, ft], I32, tag="lo32")
                lo16 = qpool.tile([P, ft], F16, tag="lo16")
                hi16 = qpool.tile([P, ft], F16, tag="hi16")
                l9 = qpool.tile([P, ft], F16, tag="l9")
                v16 = qpool.tile([P, ft], F16, tag="v16")
                # bitwise ops are not supported on Pool: always DVE
                nc.vector.tensor_scalar(out=hi32[:], in0=G32[:], scalar1=7,
                                        scalar2=None,
                                        op0=AluOp.logical_shift_right)
                nc.vector.tensor_scalar(out=lo32[:], in0=G32[:], scalar1=127,
                                        scalar2=None, op0=AluOp.bitwise_and)
                pe.tensor_copy(out=hi16[:], in_=hi32[:])
                pe.tensor_copy(out=lo16[:], in_=lo32[:])
                pe.tensor_scalar(out=l9[:], in0=L32[:], scalar1=LOSS_SCALE,
                                 scalar2=None, op0=AluOp.mult)
                pe.tensor_tensor(out=v16[:], in0=hi16[:], in1=l9[:],
                                 op=AluOp.add)

                for s in range(n_sub_t):
                    ss = slice(s * fc_t, (s + 1) * fc_t)
                    fcs = fc_t
                    prime = (t == 0 and s == 0)
                    vh_plan_s = "DDDDDDD" if prime else vh_plan
                    seed_s = "D" if prime else os.environ.get("K_VH_SEED", "P")

                    # v_hD[p, f, j] = v - j  (j = 0..HI_W-1), exact in fp16
                    v_hD = vpool.tile([P, fcs, HI_W], F16, tag="v_hD")
                    eng(seed_s).tensor_copy(
                        out=v_hD[:, :, 0:1], in_=v16[:, ss])
                    for (n, off), ch in zip(_fill_steps(HI_W), vh_plan_s):
                        e = eng(ch)
                        if e is nc.scalar:
                            e.activation(out=v_hD[:, :, off:off + n],
                                         in_=v_hD[:, :, 0:n],
                                         func=ActFn.Copy,
                                         bias=float(-off), scale=1.0)
                        else:
                            e.tensor_scalar(out=v_hD[:, :, off:off + n],
                                            in0=v_hD[:, :, 0:n],
                                            scalar1=float(-off), scalar2=None,
                                            op0=AluOp.add)

                    # lo_D[p, f, t] = lo - t (t = 0..31)
                    lo_D = lpool.tile([P, fcs, 32], F16, tag="lo_D")
                    le = eng(lod_eng)
                    le.tensor_copy(out=lo_D[:, :, 0:1], in_=lo16[:, ss])
                    for (n, off) in _fill_steps(32):
                        le.tensor_scalar(out=lo_D[:, :, off:off + n],
                                         in0=lo_D[:, :, 0:n],
                                         scalar1=float(-off), scalar2=None,
                                         op0=AluOp.add)

                    mov = bpool.tile([P, fcs, MOV_W], F16, tag="mov")
                    # C': cols 0..78
                    if prime:
                        # low-latency pipeline-fill path: C on DVE, matching
                        # the per-column conventions of the steady-state path
                        nc.vector.tensor_scalar(
                            out=mov[:, :, 0:HI_W], in0=v_hD[:],
                            scalar1=0.0, scalar2=None, op0=AluOp.is_ge)
                        if c_act > 0:
                            nc.vector.tensor_scalar(
                                out=mov[:, :, 0:c_act],
                                in0=mov[:, :, 0:c_act],
                                scalar1=2.0, scalar2=-1.0,
                                op0=AluOp.mult, op1=AluOp.add)
                    else:
                        if c_act > 0:
                            nc.scalar.activation(out=mov[:, :, 0:c_act],
                                                 in_=v_hD[:, :, 0:c_act],
                                                 func=ActFn.Sign,
                                                 bias=sign_bias[:], scale=1.0)
                        if c_act < HI_W:
                            nc.vector.tensor_scalar(
                                out=mov[:, :, c_act:HI_W],
                                in0=v_hD[:, :, c_act:HI_W],
                                scalar1=0.0, scalar2=None, op0=AluOp.is_ge)
                    # S: cols 79..157 = clamp01(v - j)
                    nc.vector.tensor_scalar(
                        out=mov[:, :, HI_W:MOV_W], in0=v_hD[:],
                        scalar1=0.0, scalar2=1.0,
                        op0=AluOp.max, op1=AluOp.min)

                    # one-hot of lo (stationary)
                    oh_lo = bpool.tile([P, fcs, P], F16, tag="oh_lo")
                    iseq_plan = os.environ.get("K_ISEQ_PLAN", "DDDD")
                    for k in range(4):
                        eng(iseq_plan[k]).tensor_scalar(
                            out=oh_lo[:, :, 32 * k:32 * (k + 1)],
                            in0=lo_D[:], scalar1=float(32 * k), scalar2=None,
                            op0=AluOp.is_equal)

                    for f in range(fcs):
                        is_last = (t == n_tiles - 1 and s == n_sub_t - 1
                                   and f == fcs - 1)
                        nc.tensor.matmul(
                            out=psum_acc[:],
                            lhsT=oh_lo[:, f, :],
                            rhs=mov[:, f, :],
                            start=first,
                            stop=is_last,
                        )
                        first = False

            # ---- cross-core AllReduce of [P, 158] partials
            acc_sb = fpool.tile([P, MOV_W], F32)
            nc.vector.tensor_copy(out=acc_sb[:], in_=psum_acc[:])
            cc_in = drpool.tile([P, MOV_W], F32)
            cc_out = drpool.tile([P, MOV_W], F32)
            nc.sync.dma_start(out=cc_in[:], in_=acc_sb[:])
            if ncores > 1:
                nc.gpsimd.collective_compute(
                    "AllReduce",
                    AluOp.add,
                    replica_groups=[list(range(ncores))],
                    ins=[cc_in.opt()],
                    outs=[cc_out.opt()],
                )
            else:
                nc.sync.dma_start(out=cc_out[:], in_=cc_in[:])
            red = fpool.tile([P, MOV_W], F32)
            nc.sync.dma_start(out=red[:], in_=cc_out[:])

            gw_sb = fpool.tile([P, HI_W], F32)
            nc.sync.dma_start(out=gw_sb[:], in_=gw_d.ap())

            # ---- decode: cntge, counts, 0.9*sums
            cn = fpool.tile([P, HI_W], F32)
            if c_act > 0:
                # sign convention: cntge = (C' + total) / 2, total = raw col0
                nc.vector.tensor_scalar(out=cn[:, 0:c_act],
                                        in0=red[:, 0:c_act],
                                        scalar1=red[:, 0:1],
                                        scalar2=0.5,
                                        op0=AluOp.add, op1=AluOp.mult)
            if c_act < HI_W:
                nc.vector.tensor_copy(out=cn[:, c_act:HI_W],
                                      in_=red[:, c_act:HI_W])

            cnts = fpool.tile([P, HI_W], F32)
            nc.vector.tensor_tensor(out=cnts[:, 0:HI_W - 1],
                                    in0=cn[:, 0:HI_W - 1],
                                    in1=cn[:, 1:HI_W], op=AluOp.subtract)
            nc.vector.tensor_copy(out=cnts[:, HI_W - 1:HI_W],
                                  in_=cn[:, HI_W - 1:HI_W])

            qs = fpool.tile([P, HI_W], F32)
            nc.vector.tensor_tensor(out=qs[:, 0:HI_W - 1],
                                    in0=red[:, HI_W:MOV_W - 1],
                                    in1=cn[:, 1:HI_W], op=AluOp.subtract)
            nc.vector.tensor_copy(out=qs[:, HI_W - 1:HI_W],
                                  in_=red[:, MOV_W - 1:MOV_W])

            # mean = qs / (0.9 * max(counts, 1))
            cnt1 = fpool.tile([P, HI_W], F32)
            nc.vector.tensor_scalar(out=cnt1[:], in0=cnts[:],
                                    scalar1=1.0, scalar2=LOSS_SCALE,
                                    op0=AluOp.max, op1=AluOp.mult)
            rcp = fpool.tile([P, HI_W], F32)
            nc.vector.reciprocal(out=rcp[:], in_=cnt1[:])
            mean = fpool.tile([P, HI_W], F32)
            nc.vector.tensor_tensor(out=mean[:], in0=qs[:], in1=rcp[:],
                                    op=AluOp.mult)
            ew = fpool.tile([P, HI_W], F32)
            nc.scalar.activation(out=ew[:], in_=mean[:], func=ActFn.Exp,
                                 scale=STEP_SIZE)
            w = fpool.tile([P, HI_W], F32)
            nc.vector.tensor_tensor(out=w[:], in0=ew[:], in1=gw_sb[:],
                                    op=AluOp.mult)
            wm = fpool.tile([P, HI_W], F32)
            nc.vector.tensor_tensor(out=wm[:], in0=w[:], in1=mean[:],
                                    op=AluOp.mult)
            pair = fpool.tile([P, 2], F32)
            nc.vector.tensor_reduce(out=pair[:, 0:1], in_=w[:],
                                    axis=mybir.AxisListType.X,
                                    op=AluOp.add)
            nc.vector.tensor_reduce(out=pair[:, 1:2], in_=wm[:],
                                    axis=mybir.AxisListType.X,
                                    op=AluOp.add)
            ones = fpool.tile([P, 1], F32)
            nc.vector.memset(ones[:], 1.0)
            psum_fin = ppool.tile([1, 2], F32, space="PSUM", tag="psum_fin")
            nc.tensor.matmul(out=psum_fin[:], lhsT=ones[:], rhs=pair[:],
                             start=True, stop=True)
            fin = fpool.tile([1, 2], F32)
            nc.vector.tensor_copy(out=fin[:], in_=psum_fin[:])
            den_r = fpool.tile([1, 1], F32)
            nc.vector.reciprocal(out=den_r[:], in_=fin[:, 0:1])
            res = fpool.tile([1, 1], F32)
            nc.vector.tensor_tensor(out=res[:], in0=fin[:, 1:2], in1=den_r[:],
                                    op=AluOp.mult)
            nc.sync.dma_start(out=out_d.ap(), in_=res[:])

    nc.compile()
    return nc


_NC_CACHE = {}


def _get_program(free=FREE, ft=FT, fc=FC):
    key = (free, ft, fc)
    if key not in _NC_CACHE:
        _NC_CACHE[key] = _build_program(free, ft, fc)
    return _NC_CACHE[key]


def _prep_inputs(losses, group_ids, group_weights, free=FREE):
    losses = np.asarray(losses, dtype=np.float32)
    group_ids = np.asarray(group_ids, dtype=np.int32)
    gw = np.asarray(group_weights, dtype=np.float32)
    n = NCORES * P * free
    l_sh = losses[:n].reshape(NCORES, P, free)
    g_sh = group_ids[:n].reshape(NCORES, P, free)
    gw_grid = np.zeros(P * HI_W, dtype=np.float32)
    gw_grid[:NUM_GROUPS] = gw
    gw_grid = np.ascontiguousarray(gw_grid.reshape(HI_W, P).T)
    in_maps = [
        {"losses": np.ascontiguousarray(l_sh[i]),
         "gids": np.ascontiguousarray(g_sh[i]),
         "gw": gw_grid}
        for i in range(NCORES)
    ]
    return in_maps


def kernel(losses, group_ids, group_weights, **run_kwargs):
    nc = _get_program()
    in_maps = _prep_inputs(losses, group_ids, group_weights)
    res = run_bass_kernel_spmd(nc, in_maps, list(range(NCORES)), **run_kwargs)
    out = np.float32(res.results[0]["out"][0, 0])
    kernel.last_results = res
    return np.array(out, dtype=np.float32)


if __name__ == "__main__":
    rng = np.random.default_rng(0)
    losses = rng.random(B, dtype=np.float32)
    gids = rng.integers(0, NUM_GROUPS, B, dtype=np.int32)
    gw = np.ones(NUM_GROUPS, dtype=np.float32) / NUM_GROUPS
    got = kernel(losses, gids, gw)
    sums = np.bincount(gids, weights=losses, minlength=NUM_GROUPS)
    cnts = np.bincount(gids, minlength=NUM_GROUPS)
    gl = np.where(cnts > 0, sums / np.maximum(cnts, 1), 0.0)
    w = gw * np.exp(STEP_SIZE * gl)
    w = w / w.sum()
    exp = float((w * gl).sum())
    print("got", got, "exp", exp, "rel", abs(got - exp) / abs(exp))


# revision 5
# speedup vs baseline: 1.0406x; 1.0406x over previous
"""GroupDRO segment-reduce kernel for 8 Trainium2 NeuronCores (v2).

Step-mask difference algorithm:
  - Shard the 2^24 elements across 8 cores (2M each), laid out [128, 16384].
  - Decompose g = hi*128 + lo (hi in [0,79), lo in [0,128)).
  - Pack v = hi + 0.9*loss in fp16 (exact integer part; loss quantized).
  - Per chunk (one SBUF column of 128 samples), build from a shared affine
    tile v_hD[j] = v - j (j = 0..78):
      S[j]  = clamp01(v - j)        -> contributes cnt{hi>j} + 0.9*loss at hi=j
      C'[j] = sign(v - j + 0.05)    -> 2*cnt{hi>=j} - total  ({-1,+1} convention)
    plus a 128-wide one-hot of lo as the stationary operand.
  - One 158-column matmul per chunk accumulates psum[lo, 0:79]  = sum oh_lo*C',
    psum[lo, 79:158] = sum oh_lo*S.
  - AllReduce the [128, 158] partials, then decode:
      cntge[j] = (C'[j] + total)/2 ;  counts[h] = cntge[h] - cntge[h+1]
      sums[h]  = (S[h] - cntge[h+1]) / 0.9
    and finish mean / exp-weight / weighted-sum on device.
"""

import os
import sys

import numpy as np

sys.path.insert(0, "/opt/trn_rl_repo")

import concourse.bacc as bacc
import concourse.tile as tile
from concourse import mybir
from concourse.bass_utils import run_bass_kernel_spmd

NUM_GROUPS = 10000
STEP_SIZE = 0.01
B = 16777216
NCORES = 8
P = 128
HI_W = 79                 # hi in [0, 79)
MOV_W = 2 * HI_W          # [C' | S]
LOSS_SCALE = 0.9
E_PER_CORE = B // NCORES  # 2,097,152
FREE = E_PER_CORE // P    # 16384 columns per core
FT = 256                  # columns per DMA tile
FC = 128                  # columns per build sub-batch

F32 = mybir.dt.float32
I32 = mybir.dt.int32
F16 = mybir.dt.float16

AluOp = mybir.AluOpType
ActFn = mybir.ActivationFunctionType


def _fill_steps(width):
    """Doubling plan covering [1, width): list of (src_width, dst_offset)."""
    steps = []
    filled = 1
    while filled < width:
        n = min(filled, width - filled)
        steps.append((n, filled))
        filled += n
    return steps


def _build_program(free=FREE, ft=FT, fc=FC, ncores=NCORES, debug=False):
    nc = bacc.Bacc("TRN2", target_bir_lowering=False, debug=debug,
                   num_devices=ncores)

    losses_d = nc.dram_tensor("losses", [P, free], F32, kind="ExternalInput")
    gids_d = nc.dram_tensor("gids", [P, free], I32, kind="ExternalInput")
    gw_d = nc.dram_tensor("gw", [P, HI_W], F32, kind="ExternalInput")
    out_d = nc.dram_tensor("out", [1, 1], F32, kind="ExternalOutput")

    n_tiles = free // ft
    n_sub = ft // fc

    # engine split knobs (sim-tuned): which engine runs each v_hD fill step
    # P=Pool(gpsimd), A=ACT(scalar), D=DVE(vector); steps are
    # (1,1),(2,2),(4,4),(8,8),(16,16),(32,32),(15,64) for width 79.
    vh_plan = os.environ.get("K_VH_PLAN", "PPPPDDA")
    lod_bufs = int(os.environ.get("K_LOD_BUFS", "1"))
    build_bufs = int(os.environ.get("K_BUILD_BUFS", "2"))
    vh_bufs = int(os.environ.get("K_VH_BUFS", "2"))
    # how many C' columns ACT computes via Sign (rest: DVE is_ge {0,1})
    c_act = int(os.environ.get("K_C_ACT", str(HI_W)))
    lod_eng = os.environ.get("K_LOD_ENG", "D")  # lo_D fill engine
    prep_eng = os.environ.get("K_PREP_ENG", "P")

    with tile.TileContext(nc) as tc:
        with (
            tc.tile_pool(name="const", bufs=1) as cpool,
            tc.tile_pool(name="inp", bufs=int(os.environ.get("K_INP_BUFS", "2"))) as ipool,
            tc.tile_pool(name="prep", bufs=int(os.environ.get("K_PREP_BUFS", "2"))) as qpool,
            tc.tile_pool(name="vh", bufs=vh_bufs) as vpool,
            tc.tile_pool(name="lod", bufs=lod_bufs) as lpool,
            tc.tile_pool(name="build", bufs=build_bufs) as bpool,
            tc.tile_pool(name="fin", bufs=1) as fpool,
            tc.tile_pool(name="psum", bufs=1, space="PSUM") as ppool,
            tc.tile_pool(name="dram", bufs=1, space="DRAM") as drpool,
        ):
            psum_acc = ppool.tile([P, MOV_W], F32, space="PSUM")

            sign_bias = cpool.tile([P, 1], F32, tag="sign_bias")
            nc.vector.memset(sign_bias[:], 0.05)

            def eng(ch):
                return {"P": nc.gpsimd, "A": nc.scalar, "D": nc.vector}[ch]

            prime_fc = int(os.environ.get("K_PRIME_FC", "48"))
            first = True
            for t in range(n_tiles):
                sl = slice(t * ft, (t + 1) * ft)
                if t == 0 and prime_fc:
                    subs = []
                    off, size = 0, prime_fc
                    while off < ft:
                        size = min(size, ft - off)
                        subs.append((off, size, off == 0))
                        off += size
                        size = min(size * 3, fc)
                else:
                    subs = [(k * fc, fc, False) for k in range(ft // fc)]
                prio = int(os.environ.get("K_PREP_PRIO", "0"))
                import contextlib
                hp = tc.high_priority(prio) if prio else contextlib.nullcontext()
                with hp:
                    L32 = ipool.tile([P, ft], F32, tag="L32")
                    G32 = ipool.tile([P, ft], I32, tag="G32")
                    nc.sync.dma_start(out=L32[:], in_=losses_d.ap()[:, sl])
                    nc.sync.dma_start(out=G32[:], in_=gids_d.ap()[:, sl])

                pe = eng("D" if t == 0 else prep_eng)
                hi32 = qpool.tile([P, ft], I32, tag="hi32")
                lo32 = qpool.tile([P, ft], I32, tag="lo32")
                lo16 = qpool.tile([P, ft], F16, tag="lo16")
                hi16 = qpool.tile([P, ft], F16, tag="hi16")
                l9 = qpool.tile([P, ft], F16, tag="l9")
                v16 = qpool.tile([P, ft], F16, tag="v16")
                hp2 = tc.high_priority(prio) if prio else contextlib.nullcontext()
                with hp2:
                    # bitwise ops are not supported on Pool: always DVE
                    nc.vector.tensor_scalar(out=hi32[:], in0=G32[:], scalar1=7,
                                            scalar2=None,
                                            op0=AluOp.logical_shift_right)
                    nc.vector.tensor_scalar(out=lo32[:], in0=G32[:],
                                            scalar1=127,
                                            scalar2=None,
                                            op0=AluOp.bitwise_and)
                    pe.tensor_copy(out=hi16[:], in_=hi32[:])
                    pe.tensor_copy(out=lo16[:], in_=lo32[:])
                    pe.tensor_scalar(out=l9[:], in0=L32[:],
                                     scalar1=LOSS_SCALE,
                                     scalar2=None, op0=AluOp.mult)
                    pe.tensor_tensor(out=v16[:], in0=hi16[:], in1=l9[:],
                                     op=AluOp.add)

                for s, (sub_off, fcs, prime) in enumerate(subs):
                    ss = slice(sub_off, sub_off + fcs)
                    vh_plan_s = "DDDDDDD" if prime else vh_plan
                    seed_s = "D" if prime else os.environ.get("K_VH_SEED", "P")

                    # v_hD[p, f, j] = v - j  (j = 0..HI_W-1), exact in fp16
                    v_hD = vpool.tile([P, fcs, HI_W], F16, tag="v_hD")
                    eng(seed_s).tensor_copy(
                        out=v_hD[:, :, 0:1], in_=v16[:, ss])
                    for (n, off), ch in zip(_fill_steps(HI_W), vh_plan_s):
                        e = eng(ch)
                        if e is nc.scalar:
                            e.activation(out=v_hD[:, :, off:off + n],
                                         in_=v_hD[:, :, 0:n],
                                         func=ActFn.Copy,
                                         bias=float(-off), scale=1.0)
                        else:
                            e.tensor_scalar(out=v_hD[:, :, off:off + n],
                                            in0=v_hD[:, :, 0:n],
                                            scalar1=float(-off), scalar2=None,
                                            op0=AluOp.add)

                    # lo_D[p, f, t] = lo - t (t = 0..31)
                    lo_D = lpool.tile([P, fcs, 32], F16, tag="lo_D")
                    le = eng(lod_eng)
                    le.tensor_copy(out=lo_D[:, :, 0:1], in_=lo16[:, ss])
                    for (n, off) in _fill_steps(32):
                        le.tensor_scalar(out=lo_D[:, :, off:off + n],
                                         in0=lo_D[:, :, 0:n],
                                         scalar1=float(-off), scalar2=None,
                                         op0=AluOp.add)

                    mov = bpool.tile([P, fcs, MOV_W], F16, tag="mov")
                    # C': cols 0..78
                    if prime:
                        # low-latency pipeline-fill path: C on DVE, matching
                        # the per-column conventions of the steady-state path
                        nc.vector.tensor_scalar(
                            out=mov[:, :, 0:HI_W], in0=v_hD[:],
                            scalar1=0.0, scalar2=None, op0=AluOp.is_ge)
                        if c_act > 0:
                            nc.vector.tensor_scalar(
                                out=mov[:, :, 0:c_act],
                                in0=mov[:, :, 0:c_act],
                                scalar1=2.0, scalar2=-1.0,
                                op0=AluOp.mult, op1=AluOp.add)
                    else:
                        if c_act > 0:
                            nc.scalar.activation(out=mov[:, :, 0:c_act],
                                                 in_=v_hD[:, :, 0:c_act],
                                                 func=ActFn.Sign,
                                                 bias=sign_bias[:], scale=1.0)
                        if c_act < HI_W:
                            nc.vector.tensor_scalar(
                                out=mov[:, :, c_act:HI_W],
                                in0=v_hD[:, :, c_act:HI_W],
                                scalar1=0.0, scalar2=None, op0=AluOp.is_ge)
                    # S: cols 79..157 = clamp01(v - j)
                    nc.vector.tensor_scalar(
                        out=mov[:, :, HI_W:MOV_W], in0=v_hD[:],
                        scalar1=0.0, scalar2=1.0,
                        op0=AluOp.max, op1=AluOp.min)

                    # one-hot of lo (stationary)
                    oh_lo = bpool.tile([P, fcs, P], F16, tag="oh_lo")
                    iseq_plan = os.environ.get("K_ISEQ_PLAN", "DDDD")
                    for k in range(4):
                        eng(iseq_plan[k]).tensor_scalar(
                            out=oh_lo[:, :, 32 * k:32 * (k + 1)],
                            in0=lo_D[:], scalar1=float(32 * k), scalar2=None,
                            op0=AluOp.is_equal)

                    for f in range(fcs):
                        is_last = (t == n_tiles - 1 and s == len(subs) - 1
                                   and f == fcs - 1)
                        nc.tensor.matmul(
                            out=psum_acc[:],
                            lhsT=oh_lo[:, f, :],
                            rhs=mov[:, f, :],
                            start=first,
                            stop=is_last,
                        )
                        first = False

            # ---- cross-core AllReduce of [P, 158] partials
            acc_sb = fpool.tile([P, MOV_W], F32)
            nc.vector.tensor_copy(out=acc_sb[:], in_=psum_acc[:])
            if ncores > 1:
                cc_in = drpool.tile([P, MOV_W], F32)
                cc_out = drpool.tile([P, MOV_W], F32)
                nc.sync.dma_start(out=cc_in[:], in_=acc_sb[:])
                nc.gpsimd.collective_compute(
                    "AllReduce",
                    AluOp.add,
                    replica_groups=[list(range(ncores))],
                    ins=[cc_in.opt()],
                    outs=[cc_out.opt()],
                )
                red = fpool.tile([P, MOV_W], F32)
                nc.sync.dma_start(out=red[:], in_=cc_out[:])
            else:
                red = acc_sb

            gw_sb = fpool.tile([P, HI_W], F32)
            nc.sync.dma_start(out=gw_sb[:], in_=gw_d.ap())

            # ---- decode: cntge, counts, 0.9*sums
            cn = fpool.tile([P, HI_W], F32)
            if c_act > 0:
                # sign convention: cntge = (C' + total) / 2, total = raw col0
                nc.vector.tensor_scalar(out=cn[:, 0:c_act],
                                        in0=red[:, 0:c_act],
                                        scalar1=red[:, 0:1],
                                        scalar2=0.5,
                                        op0=AluOp.add, op1=AluOp.mult)
            if c_act < HI_W:
                nc.vector.tensor_copy(out=cn[:, c_act:HI_W],
                                      in_=red[:, c_act:HI_W])

            cnts = fpool.tile([P, HI_W], F32)
            nc.vector.tensor_tensor(out=cnts[:, 0:HI_W - 1],
                                    in0=cn[:, 0:HI_W - 1],
                                    in1=cn[:, 1:HI_W], op=AluOp.subtract)
            nc.vector.tensor_copy(out=cnts[:, HI_W - 1:HI_W],
                                  in_=cn[:, HI_W - 1:HI_W])

            qs = fpool.tile([P, HI_W], F32)
            nc.vector.tensor_tensor(out=qs[:, 0:HI_W - 1],
                                    in0=red[:, HI_W:MOV_W - 1],
                                    in1=cn[:, 1:HI_W], op=AluOp.subtract)
            nc.vector.tensor_copy(out=qs[:, HI_W - 1:HI_W],
                                  in_=red[:, MOV_W - 1:MOV_W])

            # mean = qs / (0.9 * max(counts, 1))
            cnt1 = fpool.tile([P, HI_W], F32)
            nc.vector.tensor_scalar(out=cnt1[:], in0=cnts[:],
                                    scalar1=1.0, scalar2=LOSS_SCALE,
                                    op0=AluOp.max, op1=AluOp.mult)
            rcp = fpool.tile([P, HI_W], F32)
            nc.vector.reciprocal(out=rcp[:], in_=cnt1[:])
            mean = fpool.tile([P, HI_W], F32)
            nc.vector.tensor_tensor(out=mean[:], in0=qs[:], in1=rcp[:],
                                    op=AluOp.mult)
            ew = fpool.tile([P, HI_W], F32)
            nc.scalar.activation(out=ew[:], in_=mean[:], func=ActFn.Exp,
                                 scale=STEP_SIZE)
            w = fpool.tile([P, HI_W], F32)
            nc.vector.tensor_tensor(out=w[:], in0=ew[:], in1=gw_sb[:],
                                    op=AluOp.mult)
            wm = fpool.tile([P, HI_W], F32)
            nc.vector.tensor_tensor(out=wm[:], in0=w[:], in1=mean[:],
                                    op=AluOp.mult)
            pair = fpool.tile([P, 2], F32)
            nc.vector.tensor_reduce(out=pair[:, 0:1], in_=w[:],
                                    axis=mybir.AxisListType.X,
                                    op=AluOp.add)
            nc.vector.tensor_reduce(out=pair[:, 1:2], in_=wm[:],
                                    axis=mybir.AxisListType.X,
                                    op=AluOp.add)
            ones = fpool.tile([P, 1], F32)
            nc.vector.memset(ones[:], 1.0)
            psum_fin = ppool.tile([1, 2], F32, space="PSUM", tag="psum_fin")
            nc.tensor.matmul(out=psum_fin[:], lhsT=ones[:], rhs=pair[:],
                             start=True, stop=True)
            fin = fpool.tile([1, 2], F32)
            nc.vector.tensor_copy(out=fin[:], in_=psum_fin[:])
            den_r = fpool.tile([1, 1], F32)
            nc.vector.reciprocal(out=den_r[:], in_=fin[:, 0:1])
            res = fpool.tile([1, 1], F32)
            nc.vector.tensor_tensor(out=res[:], in0=fin[:, 1:2], in1=den_r[:],
                                    op=AluOp.mult)
            nc.sync.dma_start(out=out_d.ap(), in_=res[:])

    nc.compile()
    return nc


_NC_CACHE = {}


def _get_program(free=FREE, ft=FT, fc=FC):
    key = (free, ft, fc)
    if key not in _NC_CACHE:
        _NC_CACHE[key] = _build_program(free, ft, fc)
    return _NC_CACHE[key]


def _prep_inputs(losses, group_ids, group_weights, free=FREE):
    losses = np.asarray(losses, dtype=np.float32)
    group_ids = np.asarray(group_ids, dtype=np.int32)
    gw = np.asarray(group_weights, dtype=np.float32)
    n = NCORES * P * free
    l_sh = losses[:n].reshape(NCORES, P, free)
    g_sh = group_ids[:n].reshape(NCORES, P, free)
    gw_grid = np.zeros(P * HI_W, dtype=np.float32)
    gw_grid[:NUM_GROUPS] = gw
    gw_grid = np.ascontiguousarray(gw_grid.reshape(HI_W, P).T)
    in_maps = [
        {"losses": np.ascontiguousarray(l_sh[i]),
         "gids": np.ascontiguousarray(g_sh[i]),
         "gw": gw_grid}
        for i in range(NCORES)
    ]
    return in_maps


def kernel(losses, group_ids, group_weights, **run_kwargs):
    nc = _get_program()
    in_maps = _prep_inputs(losses, group_ids, group_weights)
    res = run_bass_kernel_spmd(nc, in_maps, list(range(NCORES)), **run_kwargs)
    out = np.float32(res.results[0]["out"][0, 0])
    kernel.last_results = res
    return np.array(out, dtype=np.float32)


if __name__ == "__main__":
    rng = np.random.default_rng(0)
    losses = rng.random(B, dtype=np.float32)
    gids = rng.integers(0, NUM_GROUPS, B, dtype=np.int32)
    gw = np.ones(NUM_GROUPS, dtype=np.float32) / NUM_GROUPS
    got = kernel(losses, gids, gw)
    sums = np.bincount(gids, weights=losses, minlength=NUM_GROUPS)
    cnts = np.bincount(gids, minlength=NUM_GROUPS)
    gl = np.where(cnts > 0, sums / np.maximum(cnts, 1), 0.0)
    w = gw * np.exp(STEP_SIZE * gl)
    w = w / w.sum()
    exp = float((w * gl).sum())
    print("got", got, "exp", exp, "rel", abs(got - exp) / abs(exp))
